# revision 1
# baseline (speedup 1.0000x reference)
"""Trainium2 Bass kernel for the gnn_message_passing problem.

Strategy (8 NeuronCores, SPMD):
  - Vocab-shard the big embedding tables (img/txt/emb) row-wise across the
    8 cores.  Each core projects its 6250-row shard (img_table @ img_W + b,
    txt_table @ txt_W + b, item = emb + 0.1*img + 0.15*txt) on the PE.
    img/txt tables and weights are cast to bf16 on the host (halves HBM
    traffic; fp32 PSUM accumulation keeps error ~1e-3).
  - AllGather the projected tables (item fp32 [N,128], vis=img|txt bf16
    [N,256]) so each core holds the full padded tables in DRAM, with row 0
    reserved as the zero pad row (matches reference's `pad()` semantics).
  - Batch-shard the rest: each core handles 64 sessions (32 pairs).  Gathers
    (indirect DMA) pull the rows for session means and h0 = itemE[inputs].
    Session masked sums are computed with block-diagonal mask matmuls
    (2 sessions packed per matmul, K=100).  All session-level [B,128] math is
    done transposed ([128, 64], feature dim on partitions) so the 128x128
    weight matmuls need no transposes and biases are per-partition.
  - The 2 hypergraph layers run per session pair with block-diagonal
    Hs / Hs^T ([100,100], built on host - pure layout), degree scaling via
    per-partition activation scales, and the session-context injection via a
    K=2 indicator matmul.
"""

import sys

sys.path.insert(0, "/opt/trn_rl_repo")

import numpy as np
import ml_dtypes

import concourse.bass as bass
import concourse.bacc as bacc
import concourse.mybir as mybir
import concourse.tile as tile
from concourse import bass_utils

BF16 = ml_dtypes.bfloat16


class Cfg:
    def __init__(self, num_node=50000, dim=128, img_dim=1000, txt_dim=768,
                 batch=512, seq=50, ncore=8, vpad=None, cc=None):
        self.N = num_node
        self.D = dim
        self.IMG = img_dim
        self.TXT = txt_dim
        self.B = batch
        self.L = seq
        self.NC = ncore
        self.VS = self.N // self.NC              # raw rows per core
        # pad shard rows up to a multiple of 128
        self.VP = vpad if vpad else ((self.VS + 127) // 128) * 128
        assert self.VP % 128 == 0
        self.NF = 1 + self.NC * self.VP          # full padded table rows
        self.BS = self.B // self.NC              # sessions per core
        self.NPAIR = self.BS // 2
        self.L2 = 2 * self.L                     # pair-packed rows (100)
        assert self.L2 <= 128
        # img k-tiles: contraction split into <=128 chunks
        self.KI = (self.IMG + 127) // 128        # 8 tiles of 125
        self.KIW = self.IMG // self.KI           # 125
        assert self.KIW * self.KI == self.IMG
        self.KT = (self.TXT + 127) // 128        # 6 tiles of 128
        self.KTW = self.TXT // self.KT
        assert self.KTW * self.KT == self.TXT
        # outer chunking of the row dimension for phase A
        if cc is None:
            cc = 7 if (self.VP // 128) % 7 == 0 else 1
        self.CC = cc
        assert (self.VP // 128) % self.CC == 0
        self.CW = self.VP // self.CC             # chunk width in rows
        self.RPC = self.CW // 128                # 128-row tiles per chunk


REAL = Cfg()


def build_program(c: Cfg):
    f32 = mybir.dt.float32
    b16 = mybir.dt.bfloat16
    i32 = mybir.dt.int32
    AF = mybir.ActivationFunctionType
    AX = mybir.AxisListType
    OP = mybir.AluOpType

    nc = bacc.Bacc("TRN2", target_bir_lowering=False, debug=False,
                   num_devices=c.NC)

    def ein(nm, sh, dt):
        return nc.dram_tensor(nm, sh, dt, kind="ExternalInput")

    imgT = ein("imgT", [c.IMG, c.VP], b16)       # img table shard, transposed
    txtT = ein("txtT", [c.TXT, c.VP], b16)
    embS = ein("embS", [c.VP, c.D], f32)
    imgW = ein("imgW", [c.IMG, c.D], b16)
    txtW = ein("txtW", [c.TXT, c.D], b16)
    imgB = ein("imgB", [1, c.D], b16)
    txtB = ein("txtB", [1, c.D], b16)
    gvW = ein("gvW", [c.D, c.D], f32)
    gvB = ein("gvB", [c.D, 1], f32)
    gtW = ein("gtW", [c.D, c.D], f32)
    gtB = ein("gtB", [c.D, 1], f32)
    q1W = ein("q1W", [c.D, c.D], f32)
    q1B = ein("q1B", [c.D, 1], f32)
    q2W = ein("q2W", [c.D, 1], f32)
    Gbd = ein("Gbd", [c.NPAIR, c.L2, c.L2], f32)
    GTbd = ein("GTbd", [c.NPAIR, c.L2, c.L2], f32)
    Mbd32 = ein("Mbd32", [c.NPAIR, c.L2, 2], f32)
    Mbd16 = ein("Mbd16", [c.NPAIR, c.L2, 2], b16)
    maskT = ein("maskT", [c.L, c.BS], f32)
    ind2 = ein("ind2", [2, c.L2], f32)
    h0idx = ein("h0idx", [c.NPAIR, c.L2, 1], i32)
    ssidx = ein("ssidx", [c.NPAIR, c.L2, 1], i32)

    outH = nc.dram_tensor("outH", [c.BS, c.L, c.D], f32, kind="ExternalOutput")

    localI = nc.dram_tensor("localI", [c.VP, c.D], f32)
    localV = nc.dram_tensor("localV", [c.VP, 2 * c.D], b16)
    Titem = nc.dram_tensor("Titem", [c.NF, c.D], f32, addr_space="Shared")
    Tvis = nc.dram_tensor("Tvis", [c.NF, 2 * c.D], b16, addr_space="Shared")

    rg = [list(range(c.NC))]

    with tile.TileContext(nc) as tc:
        with (
            tc.tile_pool(name="wpool", bufs=1) as wp,
            tc.tile_pool(name="apool", bufs=2) as ap,
            tc.tile_pool(name="ostg", bufs=4) as ost,
            tc.tile_pool(name="cbig", bufs=1) as cb,
            tc.tile_pool(name="cgat", bufs=4) as cg,
            tc.tile_pool(name="csml", bufs=3) as cs,
        ):
            # ---- constant / weight tiles ----
            wi = [wp.tile([c.KIW, c.D], b16, tag=f"wi{k}", name=f"wi{k}") for k in range(c.KI)]
            for k in range(c.KI):
                nc.sync.dma_start(wi[k][:], imgW[k * c.KIW:(k + 1) * c.KIW, :])
            wt = [wp.tile([c.KTW, c.D], b16, tag=f"wt{k}", name=f"wt{k}") for k in range(c.KT)]
            for k in range(c.KT):
                nc.sync.dma_start(wt[k][:], txtW[k * c.KTW:(k + 1) * c.KTW, :])
            bi = wp.tile([1, c.D], b16, tag="bi")
            bt = wp.tile([1, c.D], b16, tag="bt")
            nc.sync.dma_start(bi[:], imgB[:])
            nc.sync.dma_start(bt[:], txtB[:])
            ones1 = wp.tile([1, c.D], b16, tag="ones1")
            nc.vector.memset(ones1[:], 1.0)

            # ---- zero row 0 of the full tables ----
            zi = wp.tile([1, c.D], f32, tag="zi")
            zv = wp.tile([1, 2 * c.D], b16, tag="zv")
            nc.vector.memset(zi[:], 0.0)
            nc.vector.memset(zv[:], 0.0)
            nc.sync.dma_start(Titem[0:1, :], zi[:])
            nc.sync.dma_start(Tvis[0:1, :], zv[:])

            # ================= Phase A: projections =================
            imgT3 = imgT.rearrange("(k f) v -> f k v", k=c.KI)
            txtT3 = txtT.rearrange("(k f) v -> f k v", k=c.KT)
            psA_ctx = tc.tile_pool(name="psA", bufs=2, space="PSUM")
            psA = psA_ctx.__enter__()
            for ccx in range(c.CC):
                cw0 = ccx * c.CW
                ai = ap.tile([c.KIW, c.KI * c.CW], b16, tag="ai")
                nc.sync.dma_start(
                    ai[:].rearrange("f (k v) -> f k v", k=c.KI),
                    imgT3[:, :, cw0:cw0 + c.CW])
                at = ap.tile([c.KTW, c.KT * c.CW], b16, tag="at")
                nc.sync.dma_start(
                    at[:].rearrange("f (k v) -> f k v", k=c.KT),
                    txtT3[:, :, cw0:cw0 + c.CW])
                ae = ap.tile([128, c.RPC * c.D], f32, tag="ae")
                nc.sync.dma_start(
                    ae[:].rearrange("p (n d) -> p n d", n=c.RPC),
                    embS.rearrange("(n p) d -> p n d", p=128)[
                        :, ccx * c.RPC:(ccx + 1) * c.RPC, :])
                for r2 in range(c.RPC):
                    ps_i = psA.tile([128, c.D], f32, tag="psi")
                    for k in range(c.KI):
                        nc.tensor.matmul(
                            ps_i[:], lhsT=ai[:, k * c.CW + r2 * 128:
                                             k * c.CW + r2 * 128 + 128],
                            rhs=wi[k][:], start=(k == 0), stop=False)
                    nc.tensor.matmul(ps_i[:], lhsT=ones1[:], rhs=bi[:],
                                     start=False, stop=True)
                    ps_t = psA.tile([128, c.D], f32, tag="pst")
                    for k in range(c.KT):
                        nc.tensor.matmul(
                            ps_t[:], lhsT=at[:, k * c.CW + r2 * 128:
                                             k * c.CW + r2 * 128 + 128],
                            rhs=wt[k][:], start=(k == 0), stop=False)
                    nc.tensor.matmul(ps_t[:], lhsT=ones1[:], rhs=bt[:],
                                     start=False, stop=True)
                    # vis out (bf16): [128, 256] = img | txt
                    vo = ost.tile([128, 2 * c.D], b16, tag="vo")
                    nc.scalar.copy(vo[:, 0:c.D], ps_i[:])
                    nc.scalar.copy(vo[:, c.D:2 * c.D], ps_t[:])
                    # item out (fp32): emb + 0.1 img + 0.15 txt
                    t1 = ost.tile([128, c.D], f32, tag="t1")
                    nc.vector.tensor_scalar_mul(t1[:], ps_i[:], 0.1)
                    t2 = ost.tile([128, c.D], f32, tag="t2")
                    nc.vector.tensor_scalar_mul(t2[:], ps_t[:], 0.15)
                    io = ost.tile([128, c.D], f32, tag="io")
                    nc.vector.tensor_add(io[:], t1[:], t2[:])
                    nc.vector.tensor_add(
                        io[:], io[:], ae[:, r2 * c.D:(r2 + 1) * c.D])
                    row0 = cw0 + r2 * 128
                    nc.sync.dma_start(localV[row0:row0 + 128, :], vo[:])
                    nc.sync.dma_start(localI[row0:row0 + 128, :], io[:])

            psA_ctx.__exit__(None, None, None)

            # ================= Phase B: all-gather =================
            nc.gpsimd.collective_compute(
                "AllGather", mybir.AluOpType.bypass, replica_groups=rg,
                ins=[localI[:].opt()], outs=[Titem[1:c.NF, :].opt()])
            nc.gpsimd.collective_compute(
                "AllGather", mybir.AluOpType.bypass, replica_groups=rg,
                ins=[localV[:].opt()], outs=[Tvis[1:c.NF, :].opt()])

            # ================= Phase C: per-batch-shard =================
            # persistent loads
            h0all = cb.tile([c.L2, c.NPAIR * c.D], f32, tag="h0all")
            Gsb = cb.tile([c.L2, c.NPAIR * c.L2], f32, tag="Gsb")
            GTsb = cb.tile([c.L2, c.NPAIR * c.L2], f32, tag="GTsb")
            nc.sync.dma_start(
                Gsb[:].rearrange("l (p e) -> l p e", p=c.NPAIR),
                Gbd.rearrange("p l e -> l p e"))
            nc.sync.dma_start(
                GTsb[:].rearrange("l (p e) -> l p e", p=c.NPAIR),
                GTbd.rearrange("p l e -> l p e"))
            m32 = cb.tile([c.L2, c.NPAIR * 2], f32, tag="m32")
            m16 = cb.tile([c.L2, c.NPAIR * 2], b16, tag="m16")
            nc.sync.dma_start(
                m32[:].rearrange("l (p j) -> l p j", p=c.NPAIR),
                Mbd32.rearrange("p l j -> l p j"))
            nc.sync.dma_start(
                m16[:].rearrange("l (p j) -> l p j", p=c.NPAIR),
                Mbd16.rearrange("p l j -> l p j"))
            hix = cb.tile([c.L2, c.NPAIR], i32, tag="hix")
            six = cb.tile([c.L2, c.NPAIR], i32, tag="six")
            nc.sync.dma_start(hix[:], h0idx.rearrange("p l o -> l (p o)"))
            nc.sync.dma_start(six[:], ssidx.rearrange("p l o -> l (p o)"))
            mkT = cb.tile([c.L, c.BS], f32, tag="mkT")
            nc.sync.dma_start(mkT[:], maskT[:])
            i2 = cb.tile([2, c.L2], f32, tag="i2")
            nc.sync.dma_start(i2[:], ind2[:])
            ones50 = cb.tile([c.L, 1], f32, tag="ones50")
            nc.vector.memset(ones50[:], 1.0)
            # session weights
            wgv = cb.tile([c.D, c.D], f32, tag="wgv")
            wgt = cb.tile([c.D, c.D], f32, tag="wgt")
            wq1 = cb.tile([c.D, c.D], f32, tag="wq1")
            wq2 = cb.tile([c.D, 1], f32, tag="wq2")
            bgv = cb.tile([c.D, 1], f32, tag="bgv")
            bgt = cb.tile([c.D, 1], f32, tag="bgt")
            bq1 = cb.tile([c.D, 1], f32, tag="bq1")
            nc.sync.dma_start(wgv[:], gvW[:])
            nc.sync.dma_start(wgt[:], gtW[:])
            nc.sync.dma_start(wq1[:], q1W[:])
            nc.sync.dma_start(wq2[:], q2W[:])
            nc.sync.dma_start(bgv[:], gvB[:])
            nc.sync.dma_start(bgt[:], gtB[:])
            nc.sync.dma_start(bq1[:], q1B[:])

            # ---- C1: gathers + masked sums ----
            Xim = cb.tile([c.D, c.BS], f32, tag="Xim")
            Xtx = cb.tile([c.D, c.BS], f32, tag="Xtx")
            Xit = cb.tile([c.D, c.BS], f32, tag="Xit")
            with tc.tile_pool(name="psm", bufs=2, space="PSUM") as psm:
                for p in range(c.NPAIR):
                    gv_ = cg.tile([c.L2, 2 * c.D], b16, tag="gvis")
                    nc.gpsimd.indirect_dma_start(
                        out=gv_[:], out_offset=None, in_=Tvis[:],
                        in_offset=bass.IndirectOffsetOnAxis(
                            ap=six[:, p:p + 1], axis=0))
                    gi_ = cg.tile([c.L2, c.D], f32, tag="gitm")
                    nc.gpsimd.indirect_dma_start(
                        out=gi_[:], out_offset=None, in_=Titem[:],
                        in_offset=bass.IndirectOffsetOnAxis(
                            ap=six[:, p:p + 1], axis=0))
                    # h0 gather (kept in SBUF for phase C3)
                    nc.gpsimd.indirect_dma_start(
                        out=h0all[:, p * c.D:(p + 1) * c.D], out_offset=None,
                        in_=Titem[:],
                        in_offset=bass.IndirectOffsetOnAxis(
                            ap=hix[:, p:p + 1], axis=0))
                    pim = psm.tile([c.D, 2], f32, tag="pim")
                    nc.tensor.matmul(pim[:], lhsT=gv_[:, 0:c.D],
                                     rhs=m16[:, 2 * p:2 * p + 2],
                                     start=True, stop=True)
                    ptx = psm.tile([c.D, 2], f32, tag="ptx")
                    nc.tensor.matmul(ptx[:], lhsT=gv_[:, c.D:2 * c.D],
                                     rhs=m16[:, 2 * p:2 * p + 2],
                                     start=True, stop=True)
                    pit = psm.tile([c.D, 2], f32, tag="pit")
                    nc.tensor.matmul(pit[:], lhsT=gi_[:],
                                     rhs=m32[:, 2 * p:2 * p + 2],
                                     start=True, stop=True)
                    nc.scalar.copy(Xim[:, 2 * p:2 * p + 2], pim[:])
                    nc.scalar.copy(Xtx[:, 2 * p:2 * p + 2], ptx[:])
                    nc.scalar.copy(Xit[:, 2 * p:2 * p + 2], pit[:])

            # ---- C2: session fusion math (transposed [128, BS]) ----
            with (
                tc.tile_pool(name="psq", bufs=1, space="PSUM") as psq,
                tc.tile_pool(name="psg", bufs=2, space="PSUM") as psg,
            ):
                dT = psq.tile([1, c.BS], f32, tag="dT")
                nc.tensor.matmul(dT[:], lhsT=ones50[:], rhs=mkT[:],
                                 start=True, stop=True)
                invd = cs.tile([1, c.BS], f32, tag="invd")
                nc.vector.reciprocal(invd[:], dT[:])
                onesf = cb.tile([1, c.D], f32, tag="onesf")
                nc.vector.memset(onesf[:], 1.0)

                def rep_row(row):
                    # replicate a [1, BS] row across all D partitions (PSUM)
                    rp = psg.tile([c.D, c.BS], f32, tag="rep", name="rp")
                    nc.tensor.matmul(rp[:], lhsT=onesf[:], rhs=row,
                                     start=True, stop=True)
                    return rp

                Xim_m = cb.tile([c.D, c.BS], f32, tag="Xim_m")
                Xtx_m = cb.tile([c.D, c.BS], f32, tag="Xtx_m")
                Xit_m = cb.tile([c.D, c.BS], f32, tag="Xit_m")
                ir = rep_row(invd[:])
                nc.vector.tensor_tensor(Xim_m[:], Xim[:], ir[:], op=OP.mult)
                nc.vector.tensor_tensor(Xtx_m[:], Xtx[:], ir[:], op=OP.mult)
                nc.vector.tensor_tensor(Xit_m[:], Xit[:], ir[:], op=OP.mult)

                # gates on 2*session_img / 2*session_txt (scale=2 in ACT)
                pgv = psg.tile([c.D, c.BS], f32, tag="pg")
                nc.tensor.matmul(pgv[:], lhsT=wgv[:], rhs=Xim_m[:],
                                 start=True, stop=True)
                gv1 = cs.tile([c.D, c.BS], f32, tag="gv1")
                nc.scalar.activation(gv1[:], pgv[:], AF.Sigmoid,
                                     bias=bgv[:, :1], scale=2.0)
                pgt = psg.tile([c.D, c.BS], f32, tag="pg")
                nc.tensor.matmul(pgt[:], lhsT=wgt[:], rhs=Xtx_m[:],
                                 start=True, stop=True)
                gt1 = cs.tile([c.D, c.BS], f32, tag="gt1")
                nc.scalar.activation(gt1[:], pgt[:], AF.Sigmoid,
                                     bias=bgt[:, :1], scale=2.0)
                sid = cb.tile([c.D, c.BS], f32, tag="sid")
                std = cb.tile([c.D, c.BS], f32, tag="std")
                nc.vector.tensor_mul(sid[:], Xit_m[:], gv1[:])
                nc.vector.tensor_mul(std[:], Xit_m[:], gt1[:])

                # qc scores
                def qc(xin, tag):
                    pq = psg.tile([c.D, c.BS], f32, tag="pg")
                    nc.tensor.matmul(pq[:], lhsT=wq1[:], rhs=xin[:],
                                     start=True, stop=True)
                    th = cs.tile([c.D, c.BS], f32, tag="th")
                    nc.scalar.activation(th[:], pq[:], AF.Tanh,
                                         bias=bq1[:, :1], scale=1.0)
                    qq = psq.tile([1, c.BS], f32, tag="qq" + tag)
                    nc.tensor.matmul(qq[:], lhsT=wq2[:], rhs=th[:],
                                     start=True, stop=True)
                    return qq

                q1p = qc(sid, "a")
                q2p = qc(std, "b")
                q1v = cs.tile([1, c.BS], f32, tag="q1v")
                q2v = cs.tile([1, c.BS], f32, tag="q2v")
                nc.vector.tensor_copy(q1v[:], q1p[:])
                nc.vector.tensor_copy(q2v[:], q2p[:])
                qm = cs.tile([1, c.BS], f32, tag="qm")
                nc.vector.tensor_tensor(qm[:], q1v[:], q2v[:], op=OP.max)
                e1 = cs.tile([1, c.BS], f32, tag="e1")
                e2 = cs.tile([1, c.BS], f32, tag="e2")
                nc.vector.tensor_sub(e1[:], q1v[:], qm[:])
                nc.vector.tensor_sub(e2[:], q2v[:], qm[:])
                nc.scalar.activation(e1[:], e1[:], AF.Exp)
                nc.scalar.activation(e2[:], e2[:], AF.Exp)
                esum = cs.tile([1, c.BS], f32, tag="esum")
                nc.vector.tensor_add(esum[:], e1[:], e2[:])
                rsum = cs.tile([1, c.BS], f32, tag="rsum")
                nc.vector.reciprocal(rsum[:], esum[:])
                w1 = cs.tile([1, c.BS], f32, tag="w1")
                w2 = cs.tile([1, c.BS], f32, tag="w2")
                nc.vector.tensor_mul(w1[:], e1[:], rsum[:])
                nc.vector.tensor_mul(w2[:], e2[:], rsum[:])

                com = cb.tile([c.D, c.BS], f32, tag="com")
                tmp1 = cs.tile([c.D, c.BS], f32, tag="tmp1")
                w1r = rep_row(w1[:])
                nc.vector.tensor_tensor(com[:], sid[:], w1r[:], op=OP.mult)
                w2r = rep_row(w2[:])
                nc.vector.tensor_tensor(tmp1[:], std[:], w2r[:], op=OP.mult)
                nc.vector.tensor_add(com[:], com[:], tmp1[:])

                # gates on session_item
                pg2 = psg.tile([c.D, c.BS], f32, tag="pg")
                nc.tensor.matmul(pg2[:], lhsT=wgv[:], rhs=Xit_m[:],
                                 start=True, stop=True)
                gv2 = cs.tile([c.D, c.BS], f32, tag="gv2")
                nc.scalar.activation(gv2[:], pg2[:], AF.Sigmoid,
                                     bias=bgv[:, :1], scale=1.0)
                pg3 = psg.tile([c.D, c.BS], f32, tag="pg")
                nc.tensor.matmul(pg3[:], lhsT=wgt[:], rhs=Xit_m[:],
                                 start=True, stop=True)
                gt2 = cs.tile([c.D, c.BS], f32, tag="gt2")
                nc.scalar.activation(gt2[:], pg3[:], AF.Sigmoid,
                                     bias=bgt[:, :1], scale=1.0)

                sep = cs.tile([c.D, c.BS], f32, tag="sep")
                nc.vector.tensor_sub(sep[:], sid[:], com[:])
                nc.vector.tensor_mul(sep[:], gv2[:], sep[:])
                sep2 = cs.tile([c.D, c.BS], f32, tag="sep2")
                nc.vector.tensor_sub(sep2[:], std[:], com[:])
                nc.vector.tensor_mul(sep2[:], gt2[:], sep2[:])
                fus = cs.tile([c.D, c.BS], f32, tag="fus")
                nc.vector.tensor_add(fus[:], sep[:], sep2[:])
                nc.vector.tensor_add(fus[:], fus[:], com[:])
                nc.vector.tensor_scalar_mul(fus[:], fus[:], 1.0 / 3.0)
                # session_diff = item + img + txt + fusion  (transposed)
                Xs = cb.tile([c.D, c.BS], f32, tag="Xs")
                nc.vector.tensor_add(Xs[:], Xit_m[:], Xim_m[:])
                nc.vector.tensor_add(Xs[:], Xs[:], Xtx_m[:])
                nc.vector.tensor_add(Xs[:], Xs[:], fus[:])

            # identity for PE transposes
            ident = cb.tile([128, 128], f32, tag="ident")
            from concourse.masks import make_identity
            make_identity(nc, ident[:])

            # ---- C3: hypergraph layers per pair ----
            with (
                tc.tile_pool(name="psT", bufs=2, space="PSUM") as psT,
                tc.tile_pool(name="psR", bufs=2, space="PSUM") as psR,
                tc.tile_pool(name="psE", bufs=2, space="PSUM") as psE,
            ):
                for p in range(c.NPAIR):
                    Gp = Gsb[:, p * c.L2:(p + 1) * c.L2]
                    GTp = GTsb[:, p * c.L2:(p + 1) * c.L2]
                    dgn = cs.tile([c.L2, 1], f32, tag="dgn")
                    dge = cs.tile([c.L2, 1], f32, tag="dge")
                    nc.vector.reduce_sum(dgn[:], Gp, axis=AX.X)
                    nc.vector.reduce_sum(dge[:], GTp, axis=AX.X)
                    idn = cs.tile([c.L2, 1], f32, tag="idn")
                    ide = cs.tile([c.L2, 1], f32, tag="ide")
                    nc.vector.reciprocal(idn[:], dgn[:])
                    nc.vector.reciprocal(ide[:], dge[:])
                    # s pair rows: transpose Xs[:, 2p:2p+2] -> [2, 128]
                    tp = psT.tile([2, c.D], f32, tag="tp")
                    nc.tensor.transpose(tp[:], Xs[:, 2 * p:2 * p + 2],
                                        ident[:])
                    sp = cs.tile([2, c.D], f32, tag="sp")
                    nc.vector.tensor_copy(sp[:], tp[:])
                    srep = psR.tile([c.L2, c.D], f32, tag="srep")
                    nc.tensor.matmul(srep[:], lhsT=i2[:], rhs=sp[:],
                                     start=True, stop=True)
                    hcur = h0all[:, p * c.D:(p + 1) * c.D]
                    for lyr in range(2):
                        pe_ = psE.tile([c.L2, c.D], f32, tag="pe")
                        nc.tensor.matmul(pe_[:], lhsT=Gp, rhs=hcur,
                                         start=True, stop=True)
                        ee = cs.tile([c.L2, c.D], f32, tag="ee")
                        nc.scalar.activation(ee[:], pe_[:], AF.Copy,
                                             scale=ide[:, :1])
                        ph_ = psE.tile([c.L2, c.D], f32, tag="ph")
                        nc.tensor.matmul(ph_[:], lhsT=GTp, rhs=ee[:],
                                         start=True, stop=True)
                        hh = cs.tile([c.L2, c.D], f32, tag=f"hh{lyr}")
                        nc.scalar.activation(hh[:], ph_[:], AF.Copy,
                                             scale=idn[:, :1])
                        nc.vector.tensor_add(hh[:], hh[:], srep[:])
                        hcur = hh[:]
                    nc.sync.dma_start(
                        outH[2 * p:2 * p + 2].rearrange("b l d -> (b l) d"),
                        hcur)

    nc.compile()
    return nc


_CACHE = {}


def _get_program(c: Cfg):
    key = (c.N, c.B)
    if key not in _CACHE:
        _CACHE[key] = build_program(c)
    return _CACHE[key]


def _dev_row(v, c: Cfg):
    """Map a reference index (0 = pad row) to a padded-table device row."""
    v = np.asarray(v, dtype=np.int64)
    r = v - 1
    out = 1 + (r // c.VS) * c.VP + (r % c.VS)
    return np.where(v == 0, 0, out).astype(np.int32)


def _prep_inputs(c: Cfg, inputs, item, mask_item, Hs, emb_table, img_table,
                 txt_table, img_W, img_b, txt_W, txt_b, gate_v_W, gate_v_b,
                 gate_t_W, gate_t_b, qc_W1, qc_b1, qc_W2):
    f32 = np.float32
    imgT = np.zeros((c.IMG, c.NC * c.VP), dtype=BF16)
    txtT = np.zeros((c.TXT, c.NC * c.VP), dtype=BF16)
    embS = np.zeros((c.NC * c.VP, c.D), dtype=f32)
    imgTt = np.ascontiguousarray(img_table.T).astype(BF16)
    txtTt = np.ascontiguousarray(txt_table.T).astype(BF16)
    for k in range(c.NC):
        imgT[:, k * c.VP:k * c.VP + c.VS] = imgTt[:, k * c.VS:(k + 1) * c.VS]
        txtT[:, k * c.VP:k * c.VP + c.VS] = txtTt[:, k * c.VS:(k + 1) * c.VS]
        embS[k * c.VP:k * c.VP + c.VS] = emb_table[k * c.VS:(k + 1) * c.VS]

    maskf = mask_item.astype(f32)
    in_maps = []
    for k in range(c.NC):
        b0, b1 = k * c.BS, (k + 1) * c.BS
        Hk = Hs[b0:b1].astype(f32)
        mk = maskf[b0:b1]
        Gbd = np.zeros((c.NPAIR, c.L2, c.L2), f32)
        GTbd = np.zeros((c.NPAIR, c.L2, c.L2), f32)
        Mbd = np.zeros((c.NPAIR, c.L2, 2), f32)
        for p in range(c.NPAIR):
            Gbd[p, :c.L, :c.L] = Hk[2 * p]
            Gbd[p, c.L:, c.L:] = Hk[2 * p + 1]
            GTbd[p, :c.L, :c.L] = Hk[2 * p].T
            GTbd[p, c.L:, c.L:] = Hk[2 * p + 1].T
            Mbd[p, :c.L, 0] = mk[2 * p]
            Mbd[p, c.L:, 1] = mk[2 * p + 1]
        ind2 = np.zeros((2, c.L2), f32)
        ind2[0, :c.L] = 1.0
        ind2[1, c.L:] = 1.0
        in_maps.append({
            "imgT": imgT[:, k * c.VP:(k + 1) * c.VP],
            "txtT": txtT[:, k * c.VP:(k + 1) * c.VP],
            "embS": embS[k * c.VP:(k + 1) * c.VP],
            "imgW": img_W.astype(BF16),
            "txtW": txt_W.astype(BF16),
            "imgB": img_b.reshape(1, c.D).astype(BF16),
            "txtB": txt_b.reshape(1, c.D).astype(BF16),
            "gvW": gate_v_W.astype(f32), "gvB": gate_v_b.reshape(c.D, 1).astype(f32),
            "gtW": gate_t_W.astype(f32), "gtB": gate_t_b.reshape(c.D, 1).astype(f32),
            "q1W": qc_W1.astype(f32), "q1B": qc_b1.reshape(c.D, 1).astype(f32),
            "q2W": qc_W2.astype(f32),
            "Gbd": Gbd, "GTbd": GTbd,
            "Mbd32": Mbd, "Mbd16": Mbd.astype(BF16),
            "maskT": np.ascontiguousarray(mk.T),
            "ind2": ind2,
            "h0idx": _dev_row(inputs[b0:b1], c).reshape(c.NPAIR, c.L2, 1),
            "ssidx": _dev_row(item[b0:b1], c).reshape(c.NPAIR, c.L2, 1),
        })
    return in_maps


def run(c: Cfg, trace=False, **inputs):
    nc = _get_program(c)
    in_maps = _prep_inputs(c, **{k: np.asarray(v) for k, v in inputs.items()})
    res = bass_utils.run_bass_kernel_spmd(
        nc, in_maps, core_ids=list(range(c.NC)), trace=trace)
    out = np.concatenate([r["outH"] for r in res.results], axis=0)
    return out.astype(np.float32), res


def kernel(**inputs):
    out, _ = run(REAL, trace=False, **inputs)
    return out



# revision 3
# speedup vs baseline: 2.3433x; 2.3433x over previous
"""Trainium2 Bass kernel for the gnn_message_passing problem — v2.

Strategy (8 NeuronCores, SPMD, ZERO collectives):
  - Batch-shard: each core owns 64 sessions (32 pairs).
  - A combined raw table [50000, 1920] bf16 = [img(1024 padded) | txt(768) |
    emb(128)] is replicated in each core's DRAM (host prep is layout/cast
    only).  Each core gathers just the raw rows its sessions touch via
    dma_gather (SWDGE), in two index spaces:
      * item-set  (session means): non-transposed gather [rows, 1920];
        masked session sums computed by count-matrix matmuls with
        contraction over the gathered rows (no transposes needed), then
        the [64, 1920] raw sums are projected through the weights.
      * inputs-set (h0 = itemE[inputs]): TRANSPOSED gather [feat, rows]
        (feature-on-partition) which directly feeds projection matmuls
        as lhsT; pad/zero indices handled with a per-row valid mask.
  - dma_gather uses int16 indices, so the table is split in two 25000-row
    halves and host routes/sorts indices by half (slot permutation is
    absorbed into the host-built mask matrix / valid flags / h0 index map).
  - h0 rows are staged to DRAM in slot order, then one dma_gather
    (position -> slot) rebuilds the [l, pair*128] layout for the
    hypergraph layers.
  - Session fusion math (gates/qc/softmax) and the 2 hypergraph layers are
    the same transposed-[128, 64] formulation as before.
"""

import sys

sys.path.insert(0, "/opt/trn_rl_repo")

import numpy as np
import ml_dtypes

import concourse.bass as bass
import concourse.bacc as bacc
import concourse.mybir as mybir
import concourse.tile as tile
from concourse import bass_utils
from concourse._compat import cdiv

BF16 = ml_dtypes.bfloat16


class Cfg:
    def __init__(self, num_node=50000, dim=128, img_dim=1000, txt_dim=768,
                 batch=512, seq=50, ncore=8, slots_half=1792, chunk=896):
        self.N = num_node
        self.D = dim
        self.IMG = img_dim
        self.IMGP = ((img_dim + 127) // 128) * 128   # 1024
        self.TXT = txt_dim
        self.B = batch
        self.L = seq
        self.NC = ncore
        self.BS = self.B // self.NC                  # sessions per core
        self.NPAIR = self.BS // 2
        self.L2 = 2 * self.L                         # 100
        self.EL = self.IMGP + self.TXT + self.D      # 1920 table cols
        self.KC = self.EL // 128                     # 15 feature chunks
        self.KIMG = self.IMGP // 128                 # 8
        self.KTXT = self.TXT // 128                  # 6
        self.HALF = self.N // 2                      # rows per half table
        self.SH = slots_half                         # slots per half per set
        assert self.SH % 128 == 0
        self.SLOTS = 2 * self.SH                     # total slots per set
        self.NT = self.SLOTS // 128                  # row tiles per set
        self.CH = chunk                              # gather chunk (idxs)
        assert self.CH % 128 == 0 and self.SH % self.CH == 0
        self.NCH = self.SLOTS // self.CH             # gather chunks per set
        self.TPC = self.CH // 128                    # tiles per chunk
        self.NPOS = self.NPAIR * self.L2             # real positions
        self.NH = self.NPAIR * 128                   # padded h0 positions
        # aggregation feature quarters (psum-bank sized)
        self.QW = 480
        assert self.EL == 4 * self.QW


REAL = Cfg()


def build_program(c: Cfg, stage="full"):
    f32 = mybir.dt.float32
    b16 = mybir.dt.bfloat16
    i16 = mybir.dt.int16
    AF = mybir.ActivationFunctionType
    AX = mybir.AxisListType
    OP = mybir.AluOpType

    nc = bacc.Bacc("TRN2", target_bir_lowering=False, debug=False,
                   num_swdge_queues=4)

    def ein(nm, sh, dt):
        return nc.dram_tensor(nm, sh, dt, kind="ExternalInput")

    tabA = ein("tabA", [c.HALF, c.EL], b16)
    tabB = ein("tabB", [c.HALF, c.EL], b16)
    idxT = ein("idxT", [128, c.NCH * (c.CH // 16)], i16)   # item-set
    idxI = ein("idxI", [128, c.NCH * (c.CH // 16)], i16)   # inputs-set
    idxH = ein("idxH", [128, c.NH // 16], i16)             # h0 pos->slot
    Magg = ein("Magg", [128, c.NT * c.BS], b16)
    valI = ein("valI", [128, c.NT], f32)
    WHp = ein("WHp", [128, c.KC * c.D], b16)     # [0.1 Wi | 0.15 Wt | I]
    WBh = ein("WBh", [1, c.D], b16)              # 0.1 bi + 0.15 bt
    bIm01 = ein("bIm01", [1, c.D], b16)          # 0.1 bi
    bTx015 = ein("bTx015", [1, c.D], b16)        # 0.15 bt
    cntV = ein("cntV", [1, c.BS], b16)           # sum_l mask*(item!=0)
    mkT = ein("mkT", [c.L, c.BS], f32)
    i2 = ein("i2", [2, c.L2], f32)
    gvW = ein("gvW", [c.D, c.D], f32)
    gvB = ein("gvB", [c.D, 1], f32)
    gtW = ein("gtW", [c.D, c.D], f32)
    gtB = ein("gtB", [c.D, 1], f32)
    q1W = ein("q1W", [c.D, c.D], f32)
    q1B = ein("q1B", [c.D, 1], f32)
    q2W = ein("q2W", [c.D, 1], f32)
    Gin = ein("Gin", [c.L2, c.NPAIR * c.L2], f32)
    GTin = ein("GTin", [c.L2, c.NPAIR * c.L2], f32)

    outH = nc.dram_tensor("outH", [c.BS, c.L, c.D], f32, kind="ExternalOutput")
    h0stage = nc.dram_tensor("h0stage", [c.SLOTS, c.D], f32)

    with tile.TileContext(nc) as tc:
        with (
            tc.tile_pool(name="wp", bufs=1) as wp,
            tc.tile_pool(name="gp", bufs=2) as gp,
            tc.tile_pool(name="hp", bufs=4) as hp,
            tc.tile_pool(name="cs", bufs=3) as cs,
        ):
            # ---- persistent loads (plain 2D DMAs; host pre-packed) ----
            ixT = wp.tile([128, c.NCH * (c.CH // 16)], i16, tag="ixT")
            ixI = wp.tile([128, c.NCH * (c.CH // 16)], i16, tag="ixI")
            ixH = wp.tile([128, c.NH // 16], i16, tag="ixH")
            nc.sync.dma_start(ixT[:], idxT[:])
            nc.sync.dma_start(ixI[:], idxI[:])
            nc.sync.dma_start(ixH[:], idxH[:])
            mg = wp.tile([128, c.NT * c.BS], b16, tag="mg")
            nc.sync.dma_start(mg[:], Magg[:])
            vl = wp.tile([128, c.NT], f32, tag="vl")
            nc.sync.dma_start(vl[:], valI[:])
            wh = wp.tile([128, c.KC * c.D], b16, tag="wh")
            nc.sync.dma_start(wh[:], WHp[:])
            wbh = wp.tile([1, c.D], b16, tag="wbh")
            nc.sync.dma_start(wbh[:], WBh[:])
            bi01 = wp.tile([1, c.D], b16, tag="bi01")
            bt015 = wp.tile([1, c.D], b16, tag="bt015")
            cnv = wp.tile([1, c.BS], b16, tag="cnv")
            nc.sync.dma_start(bi01[:], bIm01[:])
            nc.sync.dma_start(bt015[:], bTx015[:])
            nc.sync.dma_start(cnv[:], cntV[:])
            mk = wp.tile([c.L, c.BS], f32, tag="mk")
            nc.sync.dma_start(mk[:], mkT[:])
            i2t = wp.tile([2, c.L2], f32, tag="i2t")
            nc.sync.dma_start(i2t[:], i2[:])
            wgv = wp.tile([c.D, c.D], f32, tag="wgv")
            wgt = wp.tile([c.D, c.D], f32, tag="wgt")
            wq1 = wp.tile([c.D, c.D], f32, tag="wq1")
            wq2 = wp.tile([c.D, 1], f32, tag="wq2")
            bgv = wp.tile([c.D, 1], f32, tag="bgv")
            bgt = wp.tile([c.D, 1], f32, tag="bgt")
            bq1 = wp.tile([c.D, 1], f32, tag="bq1")
            nc.sync.dma_start(wgv[:], gvW[:])
            nc.sync.dma_start(wgt[:], gtW[:])
            nc.sync.dma_start(wq1[:], q1W[:])
            nc.sync.dma_start(wq2[:], q2W[:])
            nc.sync.dma_start(bgv[:], gvB[:])
            nc.sync.dma_start(bgt[:], gtB[:])
            nc.sync.dma_start(bq1[:], q1B[:])
            Gsb = wp.tile([c.L2, c.NPAIR * c.L2], f32, tag="Gsb")
            GTsb = wp.tile([c.L2, c.NPAIR * c.L2], f32, tag="GTsb")
            nc.sync.dma_start(Gsb[:], Gin[:])
            nc.sync.dma_start(GTsb[:], GTin[:])

            onesb = wp.tile([1, 128], b16, tag="onesb")
            nc.vector.memset(onesb[:], 1.0)
            ones50 = wp.tile([c.L, 1], f32, tag="ones50")
            nc.vector.memset(ones50[:], 1.0)
            onesf = wp.tile([1, c.D], f32, tag="onesf")
            nc.vector.memset(onesf[:], 1.0)

            from concourse.masks import make_identity
            identf = wp.tile([128, 128], f32, tag="identf")
            make_identity(nc, identf[:])
            identb = wp.tile([128, 128], b16, tag="identb")
            make_identity(nc, identb[:])

            h0all = wp.tile([128, c.NPAIR, c.D], f32, tag="h0all")

            # ---- streaming phase: gathers + aggregation + h0 projection ----
            psA_ctx = tc.tile_pool(name="psA", bufs=1, space="PSUM")
            psA = psA_ctx.__enter__()
            Asum = [psA.tile([c.BS, c.QW], f32, tag=f"As{q}", name=f"As{q}")
                    for q in range(4)]
            psH_ctx = tc.tile_pool(name="psH", bufs=2, space="PSUM")
            psH = psH_ctx.__enter__()

            icols = c.CH // 16
            for ch in range(c.NCH):
                src = tabA if ch < c.NCH // 2 else tabB
                gi = gp.tile([128, c.TPC, c.EL], b16, tag="gi", name="gi")
                nc.gpsimd.dma_gather(
                    gi[:], src[:], ixT[:, ch * icols:(ch + 1) * icols],
                    c.CH, c.CH, c.EL, queue_num=ch % 4)
                gx = gp.tile([128, c.KC, c.CH], b16, tag="gx", name="gx")
                nc.gpsimd.dma_gather(
                    gx[:], src[:], ixI[:, ch * icols:(ch + 1) * icols],
                    c.CH, c.CH, c.EL, transpose=True, queue_num=(ch + 2) % 4)
                for t in range(c.TPC):
                    tg = ch * c.TPC + t
                    # masked session sums over raw rows (4 feature quarters)
                    for q in range(4):
                        nc.tensor.matmul(
                            Asum[q][:],
                            lhsT=mg[:, tg * c.BS:(tg + 1) * c.BS],
                            rhs=gi[:, t, q * c.QW:(q + 1) * c.QW],
                            start=(tg == 0), stop=(tg == c.NT - 1))
                    # h0 projection for this 128-row tile
                    ph = psH.tile([128, c.D], f32, tag="ph", name="ph")
                    for k in range(c.KC):
                        nc.tensor.matmul(
                            ph[:], lhsT=gx[:, k, t * 128:(t + 1) * 128],
                            rhs=wh[:, k * c.D:(k + 1) * c.D],
                            start=(k == 0), stop=False)
                    nc.tensor.matmul(ph[:], lhsT=onesb[:], rhs=wbh[:],
                                     start=False, stop=True)
                    hv = hp.tile([128, c.D], f32, tag="hv", name="hv")
                    nc.scalar.activation(hv[:], ph[:], AF.Copy,
                                         scale=vl[:, tg:tg + 1])
                    nc.sync.dma_start(h0stage[tg * 128:(tg + 1) * 128, :],
                                      hv[:])

            psH_ctx.__exit__(None, None, None)

            do_h0g = stage in ("h0g", "full")
            do_sums = stage in ("sums", "full")
            do_c3 = stage == "full"

            def early_out():
                zz = wp.tile([128, c.BS * c.L * c.D // 128], f32, tag="zz")
                nc.vector.memset(zz[:], 0.0)
                nc.sync.dma_start(
                    outH.rearrange("b l d -> (b l) d").rearrange(
                        "(p n) d -> p (n d)", p=128), zz[:])

            # ---- h0 gather back into [l, pair*D] layout ----
            # (chunked: a single gather's descriptor count must stay under
            #  the 1024-entry SWDGE ring carveout)
            if do_h0g:
                HCH = min(512, c.NH)
                assert c.NH % HCH == 0
                nch_h = c.NH // HCH
                ppc = HCH // 128          # pairs per chunk
                for hc in range(nch_h):
                    nc.gpsimd.dma_gather(
                        h0all[:, hc * ppc:(hc + 1) * ppc, :], h0stage[:],
                        ixH[:, hc * (HCH // 16):(hc + 1) * (HCH // 16)],
                        HCH, HCH, c.D, queue_num=hc % 4)
                if stage == "h0g":
                    nc.sync.dma_start(h0stage[0:c.L2, :], h0all[0:c.L2, 0, :])

            if not do_sums:
                psA_ctx.__exit__(None, None, None)
                early_out()
            if do_sums and not do_c3:
                early_out()

            # ---- session sums -> transposed -> projections ----
            if not do_sums:
                nc.compile()
                return nc

            sumS = wp.tile([c.BS, c.EL], b16, tag="sumS")
            for q in range(4):
                nc.scalar.copy(sumS[:, q * c.QW:(q + 1) * c.QW],
                               Asum[q][:])
            psA_ctx.__exit__(None, None, None)

            sumT = wp.tile([128, c.KC * c.BS], b16, tag="sumT")
            Simg = wp.tile([c.D, c.BS], f32, tag="Simg")
            Stxt = wp.tile([c.D, c.BS], f32, tag="Stxt")
            Sitm = wp.tile([c.D, c.BS], f32, tag="Sitm")
            with tc.tile_pool(name="psP", bufs=2, space="PSUM") as psP:
                for k in range(c.KC):
                    pt = psP.tile([128, c.BS], b16, tag="pt", name="pt")
                    nc.tensor.transpose(pt[:], sumS[:, k * 128:(k + 1) * 128],
                                        identb[0:c.BS, 0:c.BS])
                    nc.scalar.copy(sumT[:, k * c.BS:(k + 1) * c.BS], pt[:])
                pimg = psP.tile([c.D, c.BS], f32, tag="pimg", name="pimg")
                for k in range(c.KIMG):
                    nc.tensor.matmul(
                        pimg[:], lhsT=wh[:, k * c.D:(k + 1) * c.D],
                        rhs=sumT[:, k * c.BS:(k + 1) * c.BS],
                        start=(k == 0), stop=False)
                # + 0.1*img_b (x) valid-count  (bias weighted per session)
                nc.tensor.matmul(pimg[:], lhsT=bi01[:], rhs=cnv[:],
                                 start=False, stop=True)
                ptxt = psP.tile([c.D, c.BS], f32, tag="ptxt", name="ptxt")
                for k in range(c.KIMG, c.KIMG + c.KTXT):
                    nc.tensor.matmul(
                        ptxt[:], lhsT=wh[:, k * c.D:(k + 1) * c.D],
                        rhs=sumT[:, k * c.BS:(k + 1) * c.BS],
                        start=(k == c.KIMG), stop=False)
                nc.tensor.matmul(ptxt[:], lhsT=bt015[:], rhs=cnv[:],
                                 start=False, stop=True)
                # Simg = 0.1*img_sum@Wi ; Stxt = 0.15*txt_sum@Wt (pre-scaled)
                nc.scalar.copy(Simg[:], pimg[:])
                nc.scalar.copy(Stxt[:], ptxt[:])
                emb32 = cs.tile([c.D, c.BS], f32, tag="emb32")
                nc.scalar.copy(emb32[:],
                               sumT[:, (c.KC - 1) * c.BS:c.KC * c.BS])
                nc.vector.tensor_add(Sitm[:], Simg[:], Stxt[:])
                nc.vector.tensor_add(Sitm[:], Sitm[:], emb32[:])

            # ---- session fusion math (transposed [128, BS]) ----
            Xim_m = wp.tile([c.D, c.BS], f32, tag="Xim_m")
            Xtx_m = wp.tile([c.D, c.BS], f32, tag="Xtx_m")
            Xit_m = wp.tile([c.D, c.BS], f32, tag="Xit_m")
            Xs = wp.tile([c.D, c.BS], f32, tag="Xs")
            with (
                tc.tile_pool(name="psq", bufs=1, space="PSUM") as psq,
                tc.tile_pool(name="psg", bufs=2, space="PSUM") as psg,
            ):
                dT = psq.tile([1, c.BS], f32, tag="dT")
                nc.tensor.matmul(dT[:], lhsT=ones50[:], rhs=mk[:],
                                 start=True, stop=True)
                invd = cs.tile([1, c.BS], f32, tag="invd")
                nc.vector.reciprocal(invd[:], dT[:])
                invd10 = cs.tile([1, c.BS], f32, tag="invd10")
                nc.vector.tensor_scalar_mul(invd10[:], invd[:], 10.0)
                invd67 = cs.tile([1, c.BS], f32, tag="invd67")
                nc.vector.tensor_scalar_mul(invd67[:], invd[:], 1.0 / 0.15)

                def rep_row(row):
                    rp = psg.tile([c.D, c.BS], f32, tag="rep", name="rp")
                    nc.tensor.matmul(rp[:], lhsT=onesf[:], rhs=row,
                                     start=True, stop=True)
                    return rp

                tmp = cs.tile([c.D, c.BS], f32, tag="tmpm")
                r10 = rep_row(invd10[:])
                nc.vector.tensor_tensor(Xim_m[:], Simg[:], r10[:], op=OP.mult)
                r67 = rep_row(invd67[:])
                nc.vector.tensor_tensor(Xtx_m[:], Stxt[:], r67[:], op=OP.mult)
                r1 = rep_row(invd[:])
                nc.vector.tensor_tensor(Xit_m[:], Sitm[:], r1[:], op=OP.mult)

                # gates on 2*session_img / 2*session_txt (scale=2 in ACT)
                pgv = psg.tile([c.D, c.BS], f32, tag="pg", name="pgv")
                nc.tensor.matmul(pgv[:], lhsT=wgv[:], rhs=Xim_m[:],
                                 start=True, stop=True)
                gv1 = cs.tile([c.D, c.BS], f32, tag="gv1")
                nc.scalar.activation(gv1[:], pgv[:], AF.Sigmoid,
                                     bias=bgv[:, :1], scale=2.0)
                pgt = psg.tile([c.D, c.BS], f32, tag="pg", name="pgt")
                nc.tensor.matmul(pgt[:], lhsT=wgt[:], rhs=Xtx_m[:],
                                 start=True, stop=True)
                gt1 = cs.tile([c.D, c.BS], f32, tag="gt1")
                nc.scalar.activation(gt1[:], pgt[:], AF.Sigmoid,
                                     bias=bgt[:, :1], scale=2.0)
                sid = wp.tile([c.D, c.BS], f32, tag="sid")
                std = wp.tile([c.D, c.BS], f32, tag="std")
                nc.vector.tensor_mul(sid[:], Xit_m[:], gv1[:])
                nc.vector.tensor_mul(std[:], Xit_m[:], gt1[:])

                def qc(xin, tag):
                    pq = psg.tile([c.D, c.BS], f32, tag="pg", name="pq" + tag)
                    nc.tensor.matmul(pq[:], lhsT=wq1[:], rhs=xin[:],
                                     start=True, stop=True)
                    th = cs.tile([c.D, c.BS], f32, tag="th")
                    nc.scalar.activation(th[:], pq[:], AF.Tanh,
                                         bias=bq1[:, :1], scale=1.0)
                    qq = psq.tile([1, c.BS], f32, tag="qq" + tag)
                    nc.tensor.matmul(qq[:], lhsT=wq2[:], rhs=th[:],
                                     start=True, stop=True)
                    return qq

                q1p = qc(sid, "a")
                q2p = qc(std, "b")
                q1v = cs.tile([1, c.BS], f32, tag="q1v")
                q2v = cs.tile([1, c.BS], f32, tag="q2v")
                nc.vector.tensor_copy(q1v[:], q1p[:])
                nc.vector.tensor_copy(q2v[:], q2p[:])
                qm = cs.tile([1, c.BS], f32, tag="qm")
                nc.vector.tensor_tensor(qm[:], q1v[:], q2v[:], op=OP.max)
                e1 = cs.tile([1, c.BS], f32, tag="e1")
                e2 = cs.tile([1, c.BS], f32, tag="e2")
                nc.vector.tensor_sub(e1[:], q1v[:], qm[:])
                nc.vector.tensor_sub(e2[:], q2v[:], qm[:])
                nc.scalar.activation(e1[:], e1[:], AF.Exp)
                nc.scalar.activation(e2[:], e2[:], AF.Exp)
                esum = cs.tile([1, c.BS], f32, tag="esum")
                nc.vector.tensor_add(esum[:], e1[:], e2[:])
                rsum = cs.tile([1, c.BS], f32, tag="rsum")
                nc.vector.reciprocal(rsum[:], esum[:])
                w1 = cs.tile([1, c.BS], f32, tag="w1")
                w2 = cs.tile([1, c.BS], f32, tag="w2")
                nc.vector.tensor_mul(w1[:], e1[:], rsum[:])
                nc.vector.tensor_mul(w2[:], e2[:], rsum[:])

                com = wp.tile([c.D, c.BS], f32, tag="com")
                tmp1 = cs.tile([c.D, c.BS], f32, tag="tmp1")
                w1r = rep_row(w1[:])
                nc.vector.tensor_tensor(com[:], sid[:], w1r[:], op=OP.mult)
                w2r = rep_row(w2[:])
                nc.vector.tensor_tensor(tmp1[:], std[:], w2r[:], op=OP.mult)
                nc.vector.tensor_add(com[:], com[:], tmp1[:])

                # gates on session_item
                pg2 = psg.tile([c.D, c.BS], f32, tag="pg", name="pg2")
                nc.tensor.matmul(pg2[:], lhsT=wgv[:], rhs=Xit_m[:],
                                 start=True, stop=True)
                gv2 = cs.tile([c.D, c.BS], f32, tag="gv2")
                nc.scalar.activation(gv2[:], pg2[:], AF.Sigmoid,
                                     bias=bgv[:, :1], scale=1.0)
                pg3 = psg.tile([c.D, c.BS], f32, tag="pg", name="pg3")
                nc.tensor.matmul(pg3[:], lhsT=wgt[:], rhs=Xit_m[:],
                                 start=True, stop=True)
                gt2 = cs.tile([c.D, c.BS], f32, tag="gt2")
                nc.scalar.activation(gt2[:], pg3[:], AF.Sigmoid,
                                     bias=bgt[:, :1], scale=1.0)

                sep = cs.tile([c.D, c.BS], f32, tag="sep")
                nc.vector.tensor_sub(sep[:], sid[:], com[:])
                nc.vector.tensor_mul(sep[:], gv2[:], sep[:])
                sep2 = cs.tile([c.D, c.BS], f32, tag="sep2")
                nc.vector.tensor_sub(sep2[:], std[:], com[:])
                nc.vector.tensor_mul(sep2[:], gt2[:], sep2[:])
                fus = cs.tile([c.D, c.BS], f32, tag="fus")
                nc.vector.tensor_add(fus[:], sep[:], sep2[:])
                nc.vector.tensor_add(fus[:], fus[:], com[:])
                nc.vector.tensor_scalar_mul(fus[:], fus[:], 1.0 / 3.0)
                nc.vector.tensor_add(Xs[:], Xit_m[:], Xim_m[:])
                nc.vector.tensor_add(Xs[:], Xs[:], Xtx_m[:])
                nc.vector.tensor_add(Xs[:], Xs[:], fus[:])

            # ---- hypergraph layers per pair ----
            with (
                tc.tile_pool(name="psT", bufs=2, space="PSUM") as psT,
                tc.tile_pool(name="psR", bufs=2, space="PSUM") as psR,
                tc.tile_pool(name="psE", bufs=2, space="PSUM") as psE,
            ):
                for p in range(c.NPAIR):
                    Gp = Gsb[:, p * c.L2:(p + 1) * c.L2]
                    GTp = GTsb[:, p * c.L2:(p + 1) * c.L2]
                    dgn = cs.tile([c.L2, 1], f32, tag="dgn")
                    dge = cs.tile([c.L2, 1], f32, tag="dge")
                    nc.vector.reduce_sum(dgn[:], Gp, axis=AX.X)
                    nc.vector.reduce_sum(dge[:], GTp, axis=AX.X)
                    idn = cs.tile([c.L2, 1], f32, tag="idn")
                    ide = cs.tile([c.L2, 1], f32, tag="ide")
                    nc.vector.reciprocal(idn[:], dgn[:])
                    nc.vector.reciprocal(ide[:], dge[:])
                    tp = psT.tile([2, c.D], f32, tag="tp", name="tp")
                    nc.tensor.transpose(tp[:], Xs[:, 2 * p:2 * p + 2],
                                        identf[:])
                    sp = cs.tile([2, c.D], f32, tag="sp")
                    nc.vector.tensor_copy(sp[:], tp[:])
                    srep = psR.tile([c.L2, c.D], f32, tag="srep", name="srep")
                    nc.tensor.matmul(srep[:], lhsT=i2t[:], rhs=sp[:],
                                     start=True, stop=True)
                    hcur = h0all[0:c.L2, p, :]
                    for lyr in range(2):
                        pe_ = psE.tile([c.L2, c.D], f32, tag="pe", name="pe")
                        nc.tensor.matmul(pe_[:], lhsT=Gp, rhs=hcur,
                                         start=True, stop=True)
                        ee = cs.tile([c.L2, c.D], f32, tag="ee")
                        nc.scalar.activation(ee[:], pe_[:], AF.Copy,
                                             scale=ide[:, :1])
                        ph_ = psE.tile([c.L2, c.D], f32, tag="ph2", name="ph2")
                        nc.tensor.matmul(ph_[:], lhsT=GTp, rhs=ee[:],
                                         start=True, stop=True)
                        hh = cs.tile([c.L2, c.D], f32, tag=f"hh{lyr}")
                        nc.scalar.activation(hh[:], ph_[:], AF.Copy,
                                             scale=idn[:, :1])
                        nc.vector.tensor_add(hh[:], hh[:], srep[:])
                        hcur = hh[:]
                    nc.sync.dma_start(
                        outH[2 * p:2 * p + 2].rearrange("b l d -> (b l) d"),
                        hcur)

    nc.compile()
    return nc


_CACHE = {}


def _get_program(c: Cfg):
    key = (c.N, c.B, c.SH, c.CH)
    if key not in _CACHE:
        _CACHE[key] = build_program(c)
    return _CACHE[key]


def _pack_idx(v):
    """[n] int16 -> [128, n//16]: index i at partition i%16, col i//16,
    replicated across the 8 groups of 16 partitions."""
    n = len(v)
    assert n % 16 == 0
    a = np.ascontiguousarray(v.reshape(n // 16, 16).T.astype(np.int16))
    return np.tile(a, (8, 1))


def _route_slots(c: Cfg, v, keep_zero):
    """Route positions to per-half slots.

    v: [NPOS] global index values (0 = pad row semantics).
    keep_zero: if True, v==0 positions also get a slot (pointing at local
    row 0 of half A) — used for the inputs/h0 set.  If False, v==0
    positions get no slot (item set; they contribute nothing).

    Returns (loc16 [SLOTS] int16 local rows, slot_of_pos [NPOS] int32 with
    -1 for unrouted, valid [SLOTS] float32).
    """
    r = v.astype(np.int64) - 1
    selA = (v >= 1) & (r < c.HALF)
    if keep_zero:
        selA = selA | (v == 0)
    selB = (v >= 1) & (r >= c.HALF)
    posA = np.nonzero(selA)[0]
    posB = np.nonzero(selB)[0]
    if len(posA) > c.SH or len(posB) > c.SH:
        raise RuntimeError(
            f"slot overflow: {len(posA)}/{len(posB)} > {c.SH}")
    loc = np.zeros(c.SLOTS, np.int64)
    loc[:len(posA)] = np.where(v[posA] == 0, 0, r[posA])
    loc[c.SH:c.SH + len(posB)] = r[posB] - c.HALF
    slot_of_pos = np.full(len(v), -1, np.int64)
    slot_of_pos[posA] = np.arange(len(posA))
    slot_of_pos[posB] = c.SH + np.arange(len(posB))
    valid = np.zeros(c.SLOTS, np.float32)
    good = np.nonzero(v >= 1)[0]
    valid[slot_of_pos[good]] = 1.0
    return loc.astype(np.int16), slot_of_pos, valid


def _prep_inputs(c: Cfg, inputs, item, mask_item, Hs, emb_table, img_table,
                 txt_table, img_W, img_b, txt_W, txt_b, gate_v_W, gate_v_b,
                 gate_t_W, gate_t_b, qc_W1, qc_b1, qc_W2):
    f32 = np.float32

    # combined raw table [N, 1920] bf16 = [img pad 1024 | txt 768 | emb 128]
    tab = np.zeros((c.N, c.EL), dtype=BF16)
    tab[:, :c.IMG] = img_table.astype(BF16)
    tab[:, c.IMGP:c.IMGP + c.TXT] = txt_table.astype(BF16)
    tab[:, c.IMGP + c.TXT:] = emb_table.astype(BF16)
    tabA = tab[:c.HALF]
    tabB = tab[c.HALF:]

    # combined projection weights [128, 15*128] bf16 and biases
    WH = np.zeros((c.EL, c.D), f32)
    WH[:c.IMG] = 0.1 * img_W
    WH[c.IMGP:c.IMGP + c.TXT] = 0.15 * txt_W
    WH[c.IMGP + c.TXT:] = np.eye(c.D, dtype=f32)
    WHp = np.concatenate(
        [WH[k * 128:(k + 1) * 128] for k in range(c.KC)], axis=1).astype(BF16)
    bias_h0 = (0.1 * img_b + 0.15 * txt_b).astype(f32)

    maskf = mask_item.astype(f32)
    ind2 = np.zeros((2, c.L2), f32)
    ind2[0, :c.L] = 1.0
    ind2[1, c.L:] = 1.0

    in_maps = []
    for k in range(c.NC):
        b0, b1 = k * c.BS, (k + 1) * c.BS
        # pair-major position order: pos = pair*100 + l2
        item_p = item[b0:b1].reshape(c.NPAIR, c.L2).reshape(-1)
        inp_p = inputs[b0:b1].reshape(c.NPAIR, c.L2).reshape(-1)
        mask_p = maskf[b0:b1].reshape(c.NPAIR, c.L2).reshape(-1)

        # item set: slots + aggregation matrix
        locT, spT, _ = _route_slots(c, item_p, keep_zero=False)
        M = np.zeros((c.SLOTS, c.BS), f32)
        routed = spT >= 0
        pos_r = np.nonzero(routed)[0]
        sess_of_pos = 2 * (pos_r // c.L2) + (pos_r % c.L2) // c.L
        M[spT[pos_r], sess_of_pos] = mask_p[pos_r]
        Magg = np.ascontiguousarray(
            M.reshape(c.NT, 128, c.BS).transpose(1, 0, 2).reshape(
                128, c.NT * c.BS)).astype(BF16)

        # inputs set: slots + valid + h0 position map
        locI, spI, validI = _route_slots(c, inp_p, keep_zero=True)
        assert (spI >= 0).all()
        vl = np.ascontiguousarray(
            validI.reshape(c.NT, 128).T)  # [128, NT]
        hpos = np.zeros(c.NH, np.int64)
        ppos = np.arange(c.NH)
        l2 = ppos % 128
        pr = ppos // 128
        in_range = l2 < c.L2
        hpos[in_range] = spI[(pr * c.L2 + l2)[in_range]]

        Hk = Hs[b0:b1].astype(f32)
        G = np.zeros((c.NPAIR, c.L2, c.L2), f32)
        G[:, :c.L, :c.L] = Hk[0::2]
        G[:, c.L:, c.L:] = Hk[1::2]
        GT = np.ascontiguousarray(G.transpose(0, 2, 1))
        Gin = np.ascontiguousarray(
            G.transpose(1, 0, 2).reshape(c.L2, c.NPAIR * c.L2))
        GTin = np.ascontiguousarray(
            GT.transpose(1, 0, 2).reshape(c.L2, c.NPAIR * c.L2))

        in_maps.append({
            "tabA": tabA, "tabB": tabB,
            "idxT": _pack_idx(locT),
            "idxI": _pack_idx(locI),
            "idxH": _pack_idx(hpos.astype(np.int16)),
            "Magg": Magg,
            "valI": vl,
            "WHp": WHp,
            "WBh": bias_h0.reshape(1, c.D).astype(BF16),
            "bIm01": (0.1 * img_b).reshape(1, c.D).astype(BF16),
            "bTx015": (0.15 * txt_b).reshape(1, c.D).astype(BF16),
            "cntV": M.sum(0).reshape(1, c.BS).astype(BF16),
            "mkT": np.ascontiguousarray(maskf[b0:b1].T),
            "i2": ind2,
            "gvW": gate_v_W.astype(f32),
            "gvB": gate_v_b.reshape(c.D, 1).astype(f32),
            "gtW": gate_t_W.astype(f32),
            "gtB": gate_t_b.reshape(c.D, 1).astype(f32),
            "q1W": qc_W1.astype(f32), "q1B": qc_b1.reshape(c.D, 1).astype(f32),
            "q2W": qc_W2.astype(f32),
            "Gin": Gin, "GTin": GTin,
        })
    return in_maps


def run(c: Cfg, trace=False, **inputs):
    nc = _get_program(c)
    in_maps = _prep_inputs(c, **{k: np.asarray(v) for k, v in inputs.items()})
    res = bass_utils.run_bass_kernel_spmd(
        nc, in_maps, core_ids=list(range(c.NC)), trace=trace)
    out = np.concatenate([r["outH"] for r in res.results], axis=0)
    return out.astype(np.float32), res


def kernel(**inputs):
    out, _ = run(REAL, trace=False, **inputs)
    return out
